# revision 18
# baseline (speedup 1.0000x reference)
"""Disparity estimation loss kernel for Trainium2 (Bass/Tile), 8-core SPMD.

Reference computation (per pixel over the D=192 disparity axis):
    prob    = softmax(cost_volume, axis=D)
    mean    = sum(prob * d)
    var     = sum(prob * (d - mean)^2) = E[d^2] - mean^2
    logvar  = log(var + 1e-6)
Outputs: (mean [B,H,W], logvar [B,H,W]) both f32.

Strategy: shard H across 8 cores (H=256 -> 32 rows/core). All reductions are
along D which stays local. Per core, 16-h-row supergroups (2 per b):
  - Three DMA queues stream inputs concurrently (a single queue tops out
    well below the per-core HBM rate): SP HWDGE ring carries cv0 (d 0..127,
    [128, 16*512] f32, 4 MiB) of even supergroups, ACT HWDGE ring cv0 of
    odd supergroups, SWDGE the chunk1 slabs (d 128..191, two 64-partition
    slabs per supergroup, slab p = h rows h0+8p..h0+8p+8) casting f32->f16
    inline (SWDGE-only feature). All APs are simple single-level patterns
    (complex APs cost ~8us of sequencer time per HWDGE trigger). All
    triggers are hoisted to the top of each b so prefetch never queues
    behind compute.
  - exp on ScalarE -> fp16 (no max subtraction: inputs are N(0,1)), split
    per supergroup into a cv0 part and a chunk1 part so the serial tail
    after the last DMA is one small exp, not the whole supergroup.
  - TensorE matmuls contract over D: exp tile [D, 128 w-cols] stationary
    (fp16), weight columns [1, d, d^2_hi, d^2_lo] moving -> PSUM groups
    [128 w, 4]. d^2 split into exact-fp16 hi/lo bytes.
  - VectorE batched finalize (mean/var) per supergroup; PE transposes +
    mean stores deferred to end-of-b. ALL Ln's are deferred to a single
    end-of-kernel pass (var tiles for all 8 supergroups stay resident in
    SBUF): the ACT stream is then pure Exp for the whole kernel -- no
    Exp->Ln->Exp table round-trips mid-stream, so ACT never stalls on
    matmul/finalize deps and the last b's exps start the moment their
    data lands. One table switch total; the tail after the final input
    DMA is one chunk1 exp + matmuls + finalize + Ln + store (~10us)
    instead of ~40us of table-switch/stall serialization.
"""

import os
import sys

for _p in ("/opt/trn_rl_repo", "/root/.axon_site/_ro/trn_rl_repo"):
    if os.path.isdir(_p) and _p not in sys.path:
        sys.path.insert(0, _p)

import ml_dtypes
import numpy as np

import concourse.bacc as bacc
import concourse.bass as bass
import concourse.tile as tile
from concourse import mybir
from concourse.bass_utils import run_bass_kernel_spmd
from concourse.masks import make_identity

B, D, H, W = 4, 192, 256, 512
N_CORES = 8
HL = H // N_CORES  # 32 h-rows per core
F32 = mybir.dt.float32
F16 = mybir.dt.float16

# knobs (test.py may flip these before calling kernel())
TRACE = False
LAST_RESULT = None


def _make_weights() -> np.ndarray:
    """[128, 12] fp16 weight matrix; every entry is exactly representable.

    cols 0:4  -> d-chunk0 (d = row p):        [1, d, hi(d^2), lo(d^2)]  (fp16)
    cols 4:12 -> d-chunk1 (two slabs stacked on partitions):
       rows 0:64   (slab lo, d = 128+p):      [1, d, hi, lo, 0, 0, 0, 0]
       rows 64:128 (slab hi, d = 64+p):       [0, 0, 0, 0, 1, d, hi, lo]
    where hi = d^2 >> 8 (<=142), lo = d^2 & 255 — both exact in fp16.
    """
    wk = np.zeros((128, 12), dtype=np.float64)

    def cols(d):
        dsq = (d.astype(np.int64)) ** 2
        return 1.0, d, (dsq >> 8).astype(np.float64), (dsq & 255).astype(np.float64)

    p = np.arange(128, dtype=np.int64)
    wk[:, 0], wk[:, 1], wk[:, 2], wk[:, 3] = cols(p)
    c = cols(128 + p[:64])
    for k in range(4):
        wk[:64, 4 + k] = c[k]
    c = cols(64 + p[64:])
    for k in range(4):
        wk[64:, 8 + k] = c[k]
    return wk.astype(np.float16)


def build_core_kernel():
    """Build the per-core Bass module (identical program on all 8 cores)."""
    nc = bacc.Bacc("TRN2", target_bir_lowering=False, debug=False)
    x = nc.dram_tensor("x", [B, D, HL, W], F32, kind="ExternalInput")
    wk = nc.dram_tensor("wk", [128, 12], F16, kind="ExternalInput")
    mean_o = nc.dram_tensor("mean", [B, HL, W], F32, kind="ExternalOutput")
    logv_o = nc.dram_tensor("logvar", [B, HL, W], F32, kind="ExternalOutput")

    NG = 2  # supergroups per b, 16 h rows each
    GH = HL // NG  # 16
    CW = GH * W  # 8192 f32 cols of chunk0 per supergroup
    C1 = CW // 2  # 4096 cols of packed chunk1

    with tile.TileContext(nc) as tc:
        with (
            tc.tile_pool(name="cv", bufs=4) as cvp,
            tc.tile_pool(name="ex", bufs=1) as exp_p,
            tc.tile_pool(name="ex1", bufs=2) as exp1_p,
            tc.tile_pool(name="consts", bufs=1) as consts,
            tc.tile_pool(name="fin", bufs=3) as finp,
            # one var tile per supergroup, all resident until the end-of-
            # kernel Ln pass (8 x [128,64] f32 = 256B/partition each).
            # Separate tiles (not slices of one tensor) so Tile's dep
            # tracking stays exact: each Ln waits only on ITS finalize.
            tc.tile_pool(name="vars", bufs=1) as varp,
            tc.tile_pool(name="tmps", bufs=2) as tmpp,
            # outp depth 3: with 2, the DVE copy into a recycled output tile
            # waits for the PREVIOUS b's store DMA, which sits on SP behind
            # input triggers that wait on exp-fed buffer sems — a slack
            # spiral. (4 would be nicer but doesn't fit SBUF with the
            # [64,256] combined-store tiles.)
            tc.tile_pool(name="outp", bufs=3) as outp,
            tc.tile_pool(name="psum", bufs=3, space="PSUM") as psp,
            tc.tile_pool(name="pst", bufs=2, space="PSUM") as pstp,
        ):
            # consts AFTER the b0 load triggers (emitted below) so the
            # first input DMAs hit the rings at t~0; wk/ident/eps are not
            # needed until the first matmul/transpose, ~15us in.
            wkt = consts.tile([128, 12], F16, tag="wk")
            ident = consts.tile([128, 128], F32, tag="ident")
            eps_t = consts.tile([128, 1], F32, tag="eps")

            # ---- input loading. Three queues with EQUAL byte shares stream
            # concurrently (the aggregate DMA rate rises with the number of
            # simultaneously-backlogged queues — SDMA engines are per-packet
            # latency-bound): SP HWDGE ring carries the lo-slab cv0 (d
            # 0..127, h rows h0..h0+8), ACT HWDGE ring the hi-slab cv0,
            # SWDGE the chunk1 slabs, casting f32->f16 inline (SWDGE-only
            # feature; halves SBUF footprint so the cv pool can
            # quad-buffer). Every queue's share is split into ~1 MiB
            # pieces with a MATCHING exp piece (1.8us) per DMA, emitted in
            # expected-arrival order: ACT then tracks the DMA streams with
            # <=1 piece of lag instead of building a multi-group backlog
            # behind 3.6us piece-granular exps (the old tail: last exp
            # ended ~15us after the last input byte).
            Q = CW // 4  # 2048 cols = 4 h rows = 1 MiB f32

            def alloc_group():
                cv0 = cvp.tile([128, CW], F32, tag="cv0")
                c1f = cvp.tile([128, C1], F16, tag="c1f")
                return cv0, c1f

            def trig_sp(b, g, cv0):
                h0 = GH * g
                # SP: quarters qA (h0..h0+4), qB (h0+4..h0+8)
                nc.sync.dma_start(out=cv0[:, 0:Q], in_=x[b, 0:128, h0 : h0 + 4, :])
                nc.sync.dma_start(
                    out=cv0[:, Q : 2 * Q], in_=x[b, 0:128, h0 + 4 : h0 + 8, :]
                )

            def trig_act(b, g, cv0):
                h0 = GH * g
                # ACT: the full hi slab (h0+8..h0+16) as ONE 2 MiB piece.
                # ACT's triggers sit on the same sequencer as the exps, and
                # a trigger issued into a full HWDGE ring (4 in-flight)
                # BLOCKS the sequencer. With 1 DMA/group, a whole-b hoist
                # keeps ACT-ring occupancy <= 4, so ACT NEVER blocks; finer
                # ACT pieces would need mid-group triggers, which measured
                # worse (rings run shallow and the DMA window grows).
                nc.scalar.dma_start(
                    out=cv0[:, 2 * Q : 4 * Q], in_=x[b, 0:128, h0 + 8 : h0 + 16, :]
                )

            def trig_sw(b, g, c1f):
                h0 = GH * g
                # SWDGE: chunk1 (d 128..191), slab p holds h rows
                # h0+8p..h0+8p+8 on partitions 64p..64p+64; each slab split
                # into h-halves (0.5 MiB reads) ordered so the first two
                # DMAs cover ec1 cols 0:C1/2 (h-offsets 0..4 of BOTH slabs)
                for hh in range(2):
                    for p in range(2):
                        nc.gpsimd.dma_start(
                            out=c1f[
                                64 * p : 64 * p + 64,
                                hh * C1 // 2 : (hh + 1) * C1 // 2,
                            ],
                            in_=x[
                                b,
                                128:192,
                                h0 + 8 * p + 4 * hh : h0 + 8 * p + 4 * hh + 4,
                                :,
                            ],
                        )

            # wk first on the SP ring: it's 3 KiB and must not queue behind
            # 4 x 1 MiB input pieces (the first matmul needs it ~15us in)
            nc.sync.dma_start(out=wkt, in_=wk[:, :])
            pending_stores = []
            groups = [(b, g) for b in range(B) for g in range(NG)]
            tiles = {}
            # pipeline init: trigger groups 0 and 1 (4 DMAs per HWDGE ring
            # = exactly the HWDGE ring depth, so no sequencer blocking)
            for k in range(2):
                tiles[k] = alloc_group()
                trig_sp(*groups[k], tiles[k][0])
                trig_act(*groups[k], tiles[k][0])
                trig_sw(*groups[k], tiles[k][1])
            # rest of setup rides behind the init triggers on each engine
            make_identity(nc, ident)
            nc.vector.memset(eps_t, 1e-6)
            var_tiles = []
            fins = []
            for k, (b, g) in enumerate(groups):
                if g == 0:
                    # one PSUM bank per b for chunk0 sums, one for chunk1
                    bankA = psp.tile([128, 512], F32, tag="bankA")
                    bankB = psp.tile([128, 512], F32, tag="bankB")
                    fins = []
                    # hoist the NEXT b's loads: deep ring backlog keeps the
                    # SDMA engines fed (the aggregate rate depends on the
                    # number of simultaneously-backlogged queues). SP may
                    # block the Sync sequencer on ring-full — harmless,
                    # nothing compute-critical rides it; ACT's 2 triggers
                    # fit the ring and never block (see trig_act).
                    for kk in (k + 2, k + 3):
                        if kk < len(groups):
                            tiles[kk] = alloc_group()
                            trig_sp(*groups[kk], tiles[kk][0])
                            trig_act(*groups[kk], tiles[kk][0])
                            trig_sw(*groups[kk], tiles[kk][1])
                    # previous b's output stores ride the SP ring BEHIND
                    # the input triggers: their deps are long-ready so
                    # they cannot stall input prefetch.
                    for dst, t, sb in pending_stores:
                        nc.sync.dma_start(
                            out=dst[sb].rearrange("(g h) (c w) -> h c g w", g=2, c=4),
                            in_=t.rearrange("p (g w) -> p g w", g=2),
                        )
                    pending_stores = []
                cv0, c1f = tiles.pop(k)
                # exp -> fp16 in 6 pieces matching the 1 MiB DMA
                # granularity, emitted in expected-arrival order
                # [qA(SP), qC(ACT), ec1a(SWDGE), qB, qD, ec1b]: each
                # piece only waits on its own DMA(s), so ACT never
                # queues a stalled piece in front of ready work for
                # more than one piece-time.
                # separate pools: with one shared single-buffer pool, the
                # ecv0 reuse would wait on the previous group's chunk1
                # matmuls, whose SWDGE slab input is the latest-arriving
                # stream — chunk1 lateness would stall the cv0 chain.
                ecv0 = exp_p.tile([128, CW], F16, tag="ecv0")
                ec1 = exp1_p.tile([128, C1], F16, tag="ec1")

                def mm_chunk0(i_lo, i_hi, hslab):
                    # chunk0 matmuls for h rows [i_lo, i_hi) of slab
                    # hslab (0 = h0.., writes PSUM e-cols 0:4; 1 =
                    # h0+8.., e-cols 4:8)
                    for i in range(i_lo, i_hi):
                        for wc in range(4):
                            off = 8 * (32 * g + 4 * i + wc) + 4 * hslab
                            sl = slice(
                                (8 * hslab + i) * W + wc * 128,
                                (8 * hslab + i) * W + wc * 128 + 128,
                            )
                            nc.tensor.matmul(
                                bankA[:, off : off + 4],
                                ecv0[:, sl],
                                wkt[:, 0:4],
                                start=True,
                                stop=True,
                            )

                def mm_chunk1(i_lo, i_hi):
                    # chunk1 (d 128..191), both slabs at once: N=8
                    for i in range(i_lo, i_hi):
                        for wc in range(4):
                            off = 8 * (32 * g + 4 * i + wc)
                            sl = slice(i * W + wc * 128, i * W + wc * 128 + 128)
                            nc.tensor.matmul(
                                bankB[:, off : off + 8],
                                ec1[:, sl],
                                wkt[:, 4:12],
                                start=True,
                                stop=True,
                            )

                # exp pieces in expected-arrival order. Per group-period,
                # each queue delivers its 2 MiB share linearly: qA and the
                # first two slab pieces land ~mid-period, qB / qC / the
                # last slab pieces at period end. ACT consumes in-order,
                # so this order keeps its stall-in-front-of-ready-work to
                # at most one piece.
                E = mybir.ActivationFunctionType.Exp
                nc.scalar.activation(out=ecv0[:, 0:Q], in_=cv0[:, 0:Q], func=E)
                mm_chunk0(0, 4, 0)
                nc.scalar.activation(
                    out=ec1[:, 0 : C1 // 2], in_=c1f[:, 0 : C1 // 2], func=E
                )
                mm_chunk1(0, 4)
                nc.scalar.activation(out=ecv0[:, Q : 2 * Q], in_=cv0[:, Q : 2 * Q], func=E)
                mm_chunk0(4, 8, 0)
                nc.scalar.activation(
                    out=ecv0[:, 2 * Q : 4 * Q], in_=cv0[:, 2 * Q : 4 * Q], func=E
                )
                mm_chunk0(0, 8, 1)
                nc.scalar.activation(
                    out=ec1[:, C1 // 2 : C1], in_=c1f[:, C1 // 2 : C1], func=E
                )
                mm_chunk1(4, 8)

                # ---- finalize sums for this supergroup on VectorE ----
                # bank views: [128, i:8, w:4, e:8] at col 256g
                A5 = bankA[:, 256 * g : 256 * g + 256].rearrange(
                    "p (i w e) -> p i w e", i=8, w=4
                )
                # TensorTensor may read only one PSUM operand: evacuate
                # bankB's half to SBUF first, then adds read PSUM+SBUF.
                bB_sb = tmpp.tile([128, 8, 4, 8], F32, tag="bB_sb")
                nc.vector.tensor_copy(
                    bB_sb,
                    bankB[:, 256 * g : 256 * g + 256].rearrange(
                        "p (i w e) -> p i w e", i=8, w=4
                    ),
                )
                mean_sb = finp.tile([128, 64], F32, tag="mean_sb")
                # one var tile per b ([128,128]: g0 cols 0:64, g1
                # 64:128), resident until the end-of-kernel Ln pass
                if g == 0:
                    var_b = varp.tile([128, 128], F32, tag=f"var_{b}")
                    var_tiles.append(var_b)
                else:
                    var_b = var_tiles[b]
                fins.append(mean_sb)
                # dest col j3 = 4*h_local + wc = 32*half + 4i + wc
                M5 = mean_sb.rearrange("p (f i w) -> p f i w", f=2, i=8)
                V5 = var_b[:, 64 * g : 64 * g + 64].rearrange(
                    "p (f i w) -> p f i w", f=2, i=8
                )

                for half in range(2):  # 0 = lo slab (rows i), 1 = hi (8+i)
                    so = 4 * half
                    s0t = tmpp.tile([128, 8, 4], F32, tag="s0t")
                    s1t = tmpp.tile([128, 8, 4], F32, tag="s1t")
                    s2h = tmpp.tile([128, 8, 4], F32, tag="s2h")
                    s2t = tmpp.tile([128, 8, 4], F32, tag="s2t")
                    rt = tmpp.tile([128, 8, 4], F32, tag="rt")
                    m2t = tmpp.tile([128, 8, 4], F32, tag="m2t")
                    msqt = tmpp.tile([128, 8, 4], F32, tag="msqt")
                    nc.vector.tensor_add(s0t, A5[:, :, :, so + 0], bB_sb[:, :, :, so + 0])
                    nc.vector.tensor_add(s1t, A5[:, :, :, so + 1], bB_sb[:, :, :, so + 1])
                    nc.vector.tensor_add(s2h, A5[:, :, :, so + 2], bB_sb[:, :, :, so + 2])
                    nc.vector.tensor_add(s2t, A5[:, :, :, so + 3], bB_sb[:, :, :, so + 3])
                    # s2 = 256*hi + lo
                    nc.vector.scalar_tensor_tensor(
                        out=s2t,
                        in0=s2h,
                        scalar=256.0,
                        in1=s2t,
                        op0=mybir.AluOpType.mult,
                        op1=mybir.AluOpType.add,
                    )
                    nc.vector.reciprocal(rt, s0t)
                    mv = M5[:, half]
                    nc.vector.tensor_mul(mv, s1t, rt)  # mean = s1/s0
                    nc.vector.tensor_mul(m2t, s2t, rt)  # E[d^2]
                    nc.vector.tensor_mul(msqt, mv, mv)  # mean^2
                    nc.vector.tensor_sub(V5[:, half], m2t, msqt)

                if g == NG - 1:
                    # ---- end-of-b epilogue: mean transposes only (PE +
                    # DVE — nothing on ACT, which keeps streaming exps).
                    # Both groups transpose into ONE [64,256] PSUM tile ->
                    # one DVE copy -> one 4-level-AP store per b. Stores
                    # are queued for the next b's SP-ring flush (final b:
                    # after the loop). Ln's are all deferred to the end.
                    mt_ps = pstp.tile([64, 256], F32, tag="tp")
                    for gg in range(NG):
                        nc.tensor.transpose(
                            mt_ps[:, 128 * gg : 128 * gg + 128], fins[gg], ident
                        )
                    mo_sb = outp.tile([64, 256], F32, tag="mo")
                    nc.vector.tensor_copy(mo_sb, mt_ps)
                    pending_stores.append((mean_o, mo_sb, b))

            # ---- end-of-kernel logvar pass: ONE Exp->Ln table switch for
            # the whole kernel. Per-b Ln tiles [128,128] so the first 3
            # run while the last finalize is still in flight; only b3's Ln
            # waits on its g1 finalize. Each store is triggered as soon as
            # its DVE copy is emitted (SP ring is input-idle by now).
            for dst, t, sb in pending_stores:  # final b's mean store
                nc.sync.dma_start(
                    out=dst[sb].rearrange("(g h) (c w) -> h c g w", g=2, c=4),
                    in_=t.rearrange("p (g w) -> p g w", g=2),
                )
            for vb in range(B):
                lnv_sb = finp.tile([128, 128], F32, tag="lnv_sb")
                nc.scalar.activation(
                    out=lnv_sb,
                    in_=var_tiles[vb],
                    func=mybir.ActivationFunctionType.Ln,
                    bias=eps_t,
                    scale=1.0,
                )
                vt_ps = pstp.tile([64, 256], F32, tag="tp")
                for g in range(NG):
                    nc.tensor.transpose(
                        vt_ps[:, 128 * g : 128 * g + 128],
                        lnv_sb[:, 64 * g : 64 * g + 64],
                        ident,
                    )
                lo_sb = outp.tile([64, 256], F32, tag="lv")
                nc.vector.tensor_copy(lo_sb, vt_ps)
                nc.sync.dma_start(
                    out=logv_o[vb].rearrange("(g h) (c w) -> h c g w", g=2, c=4),
                    in_=lo_sb.rearrange("p (g w) -> p g w", g=2),
                )

    nc.compile()
    return nc


_NC_CACHE = None


def _get_nc():
    global _NC_CACHE
    if _NC_CACHE is None:
        _NC_CACHE = build_core_kernel()
    return _NC_CACHE


def kernel(cost_volume: np.ndarray):
    global LAST_RESULT
    cost_volume = np.ascontiguousarray(np.asarray(cost_volume, dtype=np.float32))
    assert cost_volume.shape == (B, D, H, W), cost_volume.shape

    nc = _get_nc()
    wk = _make_weights()
    in_maps = []
    for c in range(N_CORES):
        shard = np.ascontiguousarray(cost_volume[:, :, c * HL : (c + 1) * HL, :])
        in_maps.append({"x": shard, "wk": wk})

    res = run_bass_kernel_spmd(nc, in_maps, list(range(N_CORES)), trace=TRACE)
    LAST_RESULT = res

    mean = np.empty((B, H, W), dtype=np.float32)
    logv = np.empty((B, H, W), dtype=np.float32)
    for c in range(N_CORES):
        mean[:, c * HL : (c + 1) * HL, :] = res.results[c]["mean"]
        logv[:, c * HL : (c + 1) * HL, :] = res.results[c]["logvar"]
    return mean, logv



# revision 20
# speedup vs baseline: 1.0121x; 1.0121x over previous
"""Disparity estimation loss kernel for Trainium2 (Bass/Tile), 8-core SPMD.

Reference computation (per pixel over the D=192 disparity axis):
    prob    = softmax(cost_volume, axis=D)
    mean    = sum(prob * d)
    var     = sum(prob * (d - mean)^2) = E[d^2] - mean^2
    logvar  = log(var + 1e-6)
Outputs: (mean [B,H,W], logvar [B,H,W]) both f32.

Strategy: shard H across 8 cores (H=256 -> 32 rows/core). All reductions are
along D which stays local. Per core, 16-h-row supergroups (2 per b):
  - Three DMA queues stream inputs concurrently (a single queue tops out
    well below the per-core HBM rate): SP HWDGE ring carries cv0 (d 0..127,
    [128, 16*512] f32, 4 MiB) of even supergroups, ACT HWDGE ring cv0 of
    odd supergroups, SWDGE the chunk1 slabs (d 128..191, two 64-partition
    slabs per supergroup, slab p = h rows h0+8p..h0+8p+8) casting f32->f16
    inline (SWDGE-only feature). All APs are simple single-level patterns
    (complex APs cost ~8us of sequencer time per HWDGE trigger). All
    triggers are hoisted to the top of each b so prefetch never queues
    behind compute.
  - exp on ScalarE -> fp16 (no max subtraction: inputs are N(0,1)), split
    per supergroup into a cv0 part and a chunk1 part so the serial tail
    after the last DMA is one small exp, not the whole supergroup.
  - TensorE matmuls contract over D: exp tile [D, 128 w-cols] stationary
    (fp16), weight columns [1, d, d^2_hi, d^2_lo] moving -> PSUM groups
    [128 w, 4]. d^2 split into exact-fp16 hi/lo bytes.
  - VectorE batched finalize (mean/var) per supergroup; PE transposes +
    mean stores deferred to end-of-b. ALL Ln's are deferred to a single
    end-of-kernel pass (var tiles for all 8 supergroups stay resident in
    SBUF): the ACT stream is then pure Exp for the whole kernel -- no
    Exp->Ln->Exp table round-trips mid-stream, so ACT never stalls on
    matmul/finalize deps and the last b's exps start the moment their
    data lands. One table switch total; the tail after the final input
    DMA is one chunk1 exp + matmuls + finalize + Ln + store (~10us)
    instead of ~40us of table-switch/stall serialization.
"""

import os
import sys

for _p in ("/opt/trn_rl_repo", "/root/.axon_site/_ro/trn_rl_repo"):
    if os.path.isdir(_p) and _p not in sys.path:
        sys.path.insert(0, _p)

import ml_dtypes
import numpy as np

import concourse.bacc as bacc
import concourse.bass as bass
import concourse.tile as tile
from concourse import mybir
from concourse.bass_utils import run_bass_kernel_spmd
from concourse.masks import make_identity

B, D, H, W = 4, 192, 256, 512
N_CORES = 8
HL = H // N_CORES  # 32 h-rows per core
F32 = mybir.dt.float32
F16 = mybir.dt.float16

# knobs (test.py may flip these before calling kernel())
TRACE = False
LAST_RESULT = None


def _make_weights() -> np.ndarray:
    """[128, 12] fp16 weight matrix; every entry is exactly representable.

    cols 0:4  -> d-chunk0 (d = row p):        [1, d, hi(d^2), lo(d^2)]  (fp16)
    cols 4:12 -> d-chunk1 (two slabs stacked on partitions):
       rows 0:64   (slab lo, d = 128+p):      [1, d, hi, lo, 0, 0, 0, 0]
       rows 64:128 (slab hi, d = 64+p):       [0, 0, 0, 0, 1, d, hi, lo]
    where hi = d^2 >> 8 (<=142), lo = d^2 & 255 — both exact in fp16.
    """
    wk = np.zeros((128, 12), dtype=np.float64)

    def cols(d):
        dsq = (d.astype(np.int64)) ** 2
        return 1.0, d, (dsq >> 8).astype(np.float64), (dsq & 255).astype(np.float64)

    p = np.arange(128, dtype=np.int64)
    wk[:, 0], wk[:, 1], wk[:, 2], wk[:, 3] = cols(p)
    c = cols(128 + p[:64])
    for k in range(4):
        wk[:64, 4 + k] = c[k]
    c = cols(64 + p[64:])
    for k in range(4):
        wk[64:, 8 + k] = c[k]
    return wk.astype(np.float16)


def build_core_kernel():
    """Build the per-core Bass module (identical program on all 8 cores)."""
    nc = bacc.Bacc("TRN2", target_bir_lowering=False, debug=False)
    x = nc.dram_tensor("x", [B, D, HL, W], F32, kind="ExternalInput")
    wk = nc.dram_tensor("wk", [128, 12], F16, kind="ExternalInput")
    mean_o = nc.dram_tensor("mean", [B, HL, W], F32, kind="ExternalOutput")
    logv_o = nc.dram_tensor("logvar", [B, HL, W], F32, kind="ExternalOutput")

    NG = 2  # supergroups per b, 16 h rows each
    GH = HL // NG  # 16
    CW = GH * W  # 8192 f32 cols of chunk0 per supergroup
    C1 = CW // 2  # 4096 cols of packed chunk1

    with tile.TileContext(nc) as tc:
        with (
            tc.tile_pool(name="cv", bufs=4) as cvp,
            tc.tile_pool(name="ex", bufs=1) as exp_p,
            tc.tile_pool(name="ex1", bufs=2) as exp1_p,
            tc.tile_pool(name="consts", bufs=1) as consts,
            tc.tile_pool(name="fin", bufs=3) as finp,
            # one var tile per supergroup, all resident until the end-of-
            # kernel Ln pass (8 x [128,64] f32 = 256B/partition each).
            # Separate tiles (not slices of one tensor) so Tile's dep
            # tracking stays exact: each Ln waits only on ITS finalize.
            tc.tile_pool(name="vars", bufs=1) as varp,
            tc.tile_pool(name="tmps", bufs=2) as tmpp,
            # outp depth 3: with 2, the DVE copy into a recycled output tile
            # waits for the PREVIOUS b's store DMA, which sits on SP behind
            # input triggers that wait on exp-fed buffer sems — a slack
            # spiral. (4 would be nicer but doesn't fit SBUF with the
            # [64,256] combined-store tiles.)
            tc.tile_pool(name="outp", bufs=3) as outp,
            tc.tile_pool(name="psum", bufs=3, space="PSUM") as psp,
            tc.tile_pool(name="pst", bufs=2, space="PSUM") as pstp,
        ):
            # consts AFTER the b0 load triggers (emitted below) so the
            # first input DMAs hit the rings at t~0; wk/ident/eps are not
            # needed until the first matmul/transpose, ~15us in.
            wkt = consts.tile([128, 12], F16, tag="wk")
            ident = consts.tile([128, 128], F32, tag="ident")
            eps_t = consts.tile([128, 1], F32, tag="eps")

            # ---- input loading. Three queues with EQUAL byte shares stream
            # concurrently (the aggregate DMA rate rises with the number of
            # simultaneously-backlogged queues — SDMA engines are per-packet
            # latency-bound): SP HWDGE ring carries the lo-slab cv0 (d
            # 0..127, h rows h0..h0+8), ACT HWDGE ring the hi-slab cv0,
            # SWDGE the chunk1 slabs, casting f32->f16 inline (SWDGE-only
            # feature; halves SBUF footprint so the cv pool can
            # quad-buffer). Every queue's share is split into ~1 MiB
            # pieces with a MATCHING exp piece (1.8us) per DMA, emitted in
            # expected-arrival order: ACT then tracks the DMA streams with
            # <=1 piece of lag instead of building a multi-group backlog
            # behind 3.6us piece-granular exps (the old tail: last exp
            # ended ~15us after the last input byte).
            Q = CW // 4  # 2048 cols = 4 h rows = 1 MiB f32

            def alloc_group():
                cv0 = cvp.tile([128, CW], F32, tag="cv0")
                c1f = cvp.tile([128, C1], F16, tag="c1f")
                return cv0, c1f

            def trig_sp(b, g, cv0):
                h0 = GH * g
                # SP: the full lo slab (h0..h0+8) as ONE 2 MiB piece
                nc.sync.dma_start(
                    out=cv0[:, 0 : 2 * Q], in_=x[b, 0:128, h0 : h0 + 8, :]
                )

            def trig_act(b, g, cv0):
                h0 = GH * g
                # ACT: the full hi slab (h0+8..h0+16) as ONE 2 MiB piece.
                # ACT's triggers sit on the same sequencer as the exps, and
                # a trigger issued into a full HWDGE ring (4 in-flight)
                # BLOCKS the sequencer. With 1 DMA/group, a whole-b hoist
                # keeps ACT-ring occupancy <= 4, so ACT NEVER blocks; finer
                # ACT pieces would need mid-group triggers, which measured
                # worse (rings run shallow and the DMA window grows).
                nc.scalar.dma_start(
                    out=cv0[:, 2 * Q : 4 * Q], in_=x[b, 0:128, h0 + 8 : h0 + 16, :]
                )

            def trig_sw(b, g, c1f):
                h0 = GH * g
                # SWDGE: chunk1 (d 128..191), slab p: partitions 64p..64p+64
                # hold h rows h0+8p..h0+8p+8 (1 MiB read each, casting
                # f32->f16 inline — SWDGE-only feature)
                for p in range(2):
                    nc.gpsimd.dma_start(
                        out=c1f[64 * p : 64 * p + 64, :],
                        in_=x[b, 128:192, h0 + 8 * p : h0 + 8 * p + 8, :],
                    )

            # wk first on the SP ring: it's 3 KiB and must not queue behind
            # 4 x 1 MiB input pieces (the first matmul needs it ~15us in)
            nc.sync.dma_start(out=wkt, in_=wk[:, :])
            pending_stores = []
            groups = [(b, g) for b in range(B) for g in range(NG)]
            tiles = {}
            # pipeline init: trigger groups 0 and 1 (4 DMAs per HWDGE ring
            # = exactly the HWDGE ring depth, so no sequencer blocking)
            for k in range(2):
                tiles[k] = alloc_group()
                trig_sp(*groups[k], tiles[k][0])
                trig_act(*groups[k], tiles[k][0])
                trig_sw(*groups[k], tiles[k][1])
            # rest of setup rides behind the init triggers on each engine
            make_identity(nc, ident)
            nc.vector.memset(eps_t, 1e-6)
            var_tiles = []
            fins = []
            for k, (b, g) in enumerate(groups):
                if g == 0:
                    # one PSUM bank per b for chunk0 sums, one for chunk1
                    bankA = psp.tile([128, 512], F32, tag="bankA")
                    bankB = psp.tile([128, 512], F32, tag="bankB")
                    fins = []
                    # hoist the NEXT b's loads: deep ring backlog keeps the
                    # SDMA engines fed (the aggregate rate depends on the
                    # number of simultaneously-backlogged queues). SP may
                    # block the Sync sequencer on ring-full — harmless,
                    # nothing compute-critical rides it; ACT's 2 triggers
                    # fit the ring and never block (see trig_act).
                    for kk in (k + 2, k + 3):
                        if kk < len(groups):
                            tiles[kk] = alloc_group()
                            trig_sp(*groups[kk], tiles[kk][0])
                            trig_act(*groups[kk], tiles[kk][0])
                            trig_sw(*groups[kk], tiles[kk][1])
                    # previous b's output stores ride the SP ring BEHIND
                    # the input triggers: their deps are long-ready so
                    # they cannot stall input prefetch.
                    for dst, t, sb in pending_stores:
                        nc.sync.dma_start(
                            out=dst[sb].rearrange("(g h) (c w) -> h c g w", g=2, c=4),
                            in_=t.rearrange("p (g w) -> p g w", g=2),
                        )
                    pending_stores = []
                cv0, c1f = tiles.pop(k)
                # exp -> fp16 in 6 pieces matching the 1 MiB DMA
                # granularity, emitted in expected-arrival order
                # [qA(SP), qC(ACT), ec1a(SWDGE), qB, qD, ec1b]: each
                # piece only waits on its own DMA(s), so ACT never
                # queues a stalled piece in front of ready work for
                # more than one piece-time.
                # separate pools: with one shared single-buffer pool, the
                # ecv0 reuse would wait on the previous group's chunk1
                # matmuls, whose SWDGE slab input is the latest-arriving
                # stream — chunk1 lateness would stall the cv0 chain.
                ecv0 = exp_p.tile([128, CW], F16, tag="ecv0")
                ec1 = exp1_p.tile([128, C1], F16, tag="ec1")

                def mm_chunk0(i_lo, i_hi, hslab):
                    # chunk0 matmuls for h rows [i_lo, i_hi) of slab
                    # hslab (0 = h0.., writes PSUM e-cols 0:4; 1 =
                    # h0+8.., e-cols 4:8)
                    for i in range(i_lo, i_hi):
                        for wc in range(4):
                            off = 8 * (32 * g + 4 * i + wc) + 4 * hslab
                            sl = slice(
                                (8 * hslab + i) * W + wc * 128,
                                (8 * hslab + i) * W + wc * 128 + 128,
                            )
                            nc.tensor.matmul(
                                bankA[:, off : off + 4],
                                ecv0[:, sl],
                                wkt[:, 0:4],
                                start=True,
                                stop=True,
                            )

                def mm_chunk1(i_lo, i_hi):
                    # chunk1 (d 128..191), both slabs at once: N=8
                    for i in range(i_lo, i_hi):
                        for wc in range(4):
                            off = 8 * (32 * g + 4 * i + wc)
                            sl = slice(i * W + wc * 128, i * W + wc * 128 + 128)
                            nc.tensor.matmul(
                                bankB[:, off : off + 8],
                                ec1[:, sl],
                                wkt[:, 4:12],
                                start=True,
                                stop=True,
                            )

                # exp parts match DMA granularity: each part only waits on
                # its own DMA (lo on SP, hi on ACT, ec1 on SWDGE slabs)
                E = mybir.ActivationFunctionType.Exp
                nc.scalar.activation(out=ecv0[:, 0 : 2 * Q], in_=cv0[:, 0 : 2 * Q], func=E)
                mm_chunk0(0, 8, 0)
                nc.scalar.activation(
                    out=ecv0[:, 2 * Q : 4 * Q], in_=cv0[:, 2 * Q : 4 * Q], func=E
                )
                mm_chunk0(0, 8, 1)
                nc.scalar.activation(out=ec1, in_=c1f, func=E)
                mm_chunk1(0, 8)

                # ---- finalize sums for this supergroup on VectorE ----
                # bank views: [128, i:8, w:4, e:8] at col 256g
                A5 = bankA[:, 256 * g : 256 * g + 256].rearrange(
                    "p (i w e) -> p i w e", i=8, w=4
                )
                # TensorTensor may read only one PSUM operand: evacuate
                # bankB's half to SBUF first, then adds read PSUM+SBUF.
                bB_sb = tmpp.tile([128, 8, 4, 8], F32, tag="bB_sb")
                nc.vector.tensor_copy(
                    bB_sb,
                    bankB[:, 256 * g : 256 * g + 256].rearrange(
                        "p (i w e) -> p i w e", i=8, w=4
                    ),
                )
                mean_sb = finp.tile([128, 64], F32, tag="mean_sb")
                # one var tile per b ([128,128]: g0 cols 0:64, g1
                # 64:128), resident until the end-of-kernel Ln pass
                if g == 0:
                    var_b = varp.tile([128, 128], F32, tag=f"var_{b}")
                    var_tiles.append(var_b)
                else:
                    var_b = var_tiles[b]
                fins.append(mean_sb)
                # dest col j3 = 4*h_local + wc = 32*half + 4i + wc
                M5 = mean_sb.rearrange("p (f i w) -> p f i w", f=2, i=8)
                V5 = var_b[:, 64 * g : 64 * g + 64].rearrange(
                    "p (f i w) -> p f i w", f=2, i=8
                )

                for half in range(2):  # 0 = lo slab (rows i), 1 = hi (8+i)
                    so = 4 * half
                    s0t = tmpp.tile([128, 8, 4], F32, tag="s0t")
                    s1t = tmpp.tile([128, 8, 4], F32, tag="s1t")
                    s2h = tmpp.tile([128, 8, 4], F32, tag="s2h")
                    s2t = tmpp.tile([128, 8, 4], F32, tag="s2t")
                    rt = tmpp.tile([128, 8, 4], F32, tag="rt")
                    m2t = tmpp.tile([128, 8, 4], F32, tag="m2t")
                    msqt = tmpp.tile([128, 8, 4], F32, tag="msqt")
                    nc.vector.tensor_add(s0t, A5[:, :, :, so + 0], bB_sb[:, :, :, so + 0])
                    nc.vector.tensor_add(s1t, A5[:, :, :, so + 1], bB_sb[:, :, :, so + 1])
                    nc.vector.tensor_add(s2h, A5[:, :, :, so + 2], bB_sb[:, :, :, so + 2])
                    nc.vector.tensor_add(s2t, A5[:, :, :, so + 3], bB_sb[:, :, :, so + 3])
                    # s2 = 256*hi + lo
                    nc.vector.scalar_tensor_tensor(
                        out=s2t,
                        in0=s2h,
                        scalar=256.0,
                        in1=s2t,
                        op0=mybir.AluOpType.mult,
                        op1=mybir.AluOpType.add,
                    )
                    nc.vector.reciprocal(rt, s0t)
                    mv = M5[:, half]
                    nc.vector.tensor_mul(mv, s1t, rt)  # mean = s1/s0
                    nc.vector.tensor_mul(m2t, s2t, rt)  # E[d^2]
                    nc.vector.tensor_mul(msqt, mv, mv)  # mean^2
                    nc.vector.tensor_sub(V5[:, half], m2t, msqt)

                if g == NG - 1:
                    # ---- end-of-b epilogue: mean transposes only (PE +
                    # DVE — nothing on ACT, which keeps streaming exps).
                    # Both groups transpose into ONE [64,256] PSUM tile ->
                    # one DVE copy -> one 4-level-AP store per b. Stores
                    # are queued for the next b's SP-ring flush (final b:
                    # after the loop). Ln's are all deferred to the end.
                    mt_ps = pstp.tile([64, 256], F32, tag="tp")
                    for gg in range(NG):
                        nc.tensor.transpose(
                            mt_ps[:, 128 * gg : 128 * gg + 128], fins[gg], ident
                        )
                    mo_sb = outp.tile([64, 256], F32, tag="mo")
                    nc.vector.tensor_copy(mo_sb, mt_ps)
                    pending_stores.append((mean_o, mo_sb, b))

            # ---- end-of-kernel logvar pass: ONE Exp->Ln table switch for
            # the whole kernel. Per-b Ln tiles [128,128] so the first 3
            # run while the last finalize is still in flight; only b3's Ln
            # waits on its g1 finalize. Each store is triggered as soon as
            # its DVE copy is emitted (SP ring is input-idle by now).
            for dst, t, sb in pending_stores:  # final b's mean store
                nc.sync.dma_start(
                    out=dst[sb].rearrange("(g h) (c w) -> h c g w", g=2, c=4),
                    in_=t.rearrange("p (g w) -> p g w", g=2),
                )
            for vb in range(B):
                lnv_sb = finp.tile([128, 128], F32, tag="lnv_sb")
                nc.scalar.activation(
                    out=lnv_sb,
                    in_=var_tiles[vb],
                    func=mybir.ActivationFunctionType.Ln,
                    bias=eps_t,
                    scale=1.0,
                )
                vt_ps = pstp.tile([64, 256], F32, tag="tp")
                for g in range(NG):
                    nc.tensor.transpose(
                        vt_ps[:, 128 * g : 128 * g + 128],
                        lnv_sb[:, 64 * g : 64 * g + 64],
                        ident,
                    )
                lo_sb = outp.tile([64, 256], F32, tag="lv")
                nc.vector.tensor_copy(lo_sb, vt_ps)
                nc.sync.dma_start(
                    out=logv_o[vb].rearrange("(g h) (c w) -> h c g w", g=2, c=4),
                    in_=lo_sb.rearrange("p (g w) -> p g w", g=2),
                )

    nc.compile()
    return nc


_NC_CACHE = None


def _get_nc():
    global _NC_CACHE
    if _NC_CACHE is None:
        _NC_CACHE = build_core_kernel()
    return _NC_CACHE


def kernel(cost_volume: np.ndarray):
    global LAST_RESULT
    cost_volume = np.ascontiguousarray(np.asarray(cost_volume, dtype=np.float32))
    assert cost_volume.shape == (B, D, H, W), cost_volume.shape

    nc = _get_nc()
    wk = _make_weights()
    in_maps = []
    for c in range(N_CORES):
        shard = np.ascontiguousarray(cost_volume[:, :, c * HL : (c + 1) * HL, :])
        in_maps.append({"x": shard, "wk": wk})

    res = run_bass_kernel_spmd(nc, in_maps, list(range(N_CORES)), trace=TRACE)
    LAST_RESULT = res

    mean = np.empty((B, H, W), dtype=np.float32)
    logv = np.empty((B, H, W), dtype=np.float32)
    for c in range(N_CORES):
        mean[:, c * HL : (c + 1) * HL, :] = res.results[c]["mean"]
        logv[:, c * HL : (c + 1) * HL, :] = res.results[c]["logvar"]
    return mean, logv

